# revision 1
# baseline (speedup 1.0000x reference)
"""BinaryLinear forward for Trainium2, 8-core SPMD.

Reference computation (per problem):
    scale = mean(|W|)                    # scalar over full W
    out   = x @ (sign(W) * scale).T      # x [8, 2048, 4096], W [4096, 4096]

Sharding: data-parallel over the leading batch dim (8 batches -> 8 cores).
Each core computes out_b = x_b @ (sign(W) * scale).T with x_b [2048, 4096]
and the full W replicated. The scalar `scale` needs all of W; each core
reduces |W| over its own 1/8 row-slice (passed as an extra sharded input so
the access pattern stays static) and an AllReduce combines the partials.

Device algorithm per core:
  - x tiles are cast fp32->bf16 during DMA (SWDGE cast), then transposed
    SBUF->SBUF with the DMA xbar into XT k-major tiles (lhsT layout).
  - W row-tiles are cast-DMA'd to bf16, xbar-transposed into WT chunks
    (rhs layout), then sign() is applied in place on the ACT engine.
    sign(W)*scale is folded as: matmul against sign(W) (exact +-1 in bf16),
    scale applied on the psum->SBUF eviction (ACT copy with per-partition
    scale). bf16 is near-lossless here: sign is exact, only x rounds.
  - PE does K=128-deep accumulating matmuls: psum[m=128, n=256] over 32
    k-tiles, lhsT = XT[mt] slice, rhs = WT chunk slice.
"""

import numpy as np

P = 128
M = 2048  # rows per core (one batch)
K = 4096  # in_features
N = 4096  # out_features
MT = M // P  # 16 m-tiles
KT = K // P  # 32 k-tiles
CH = 256  # out_features per chunk
NCH = N // CH  # 16 chunks
RSLICE = N // 8  # 512 rows of W reduced per core

_CACHE = {}


def _build_program(
    n_cores,
    reps=1,
    stage_bufs=3,
    wt_bufs=2,
    psum_bufs=7,
    out_bufs=4,
    no_matmul=False,
    no_wstream=False,
    no_xprep=False,
    no_store=False,
    no_sign=False,
    no_scale=False,
    mm_const_rhs=False,
    sign_as_copy=False,
    sign_on_dve=False,
):
    import concourse.bass as bass
    import concourse.mybir as mybir
    import concourse.tile as tile
    from concourse import bacc

    fp32 = mybir.dt.float32
    bf16 = mybir.dt.bfloat16

    nc = bacc.Bacc("TRN2", target_bir_lowering=False, debug=False, num_devices=n_cores)

    x_d = nc.dram_tensor("x", [M, K], fp32, kind="ExternalInput").ap()
    w_d = nc.dram_tensor("weight", [N, K], fp32, kind="ExternalInput").ap()
    ws_d = nc.dram_tensor("wslice", [RSLICE, K], fp32, kind="ExternalInput").ap()
    o_d = nc.dram_tensor("out", [M, N], fp32, kind="ExternalOutput").ap()

    with tile.TileContext(nc) as tc:
        cc_in, cc_in_free = tc.tile(
            [P, 1], fp32, space=bass.MemorySpace.DRAM, name="cc_in"
        )
        cc_out, cc_out_free = tc.tile(
            [P, 1],
            fp32,
            space=bass.MemorySpace.DRAM,
            addr_space="Shared",
            name="cc_out",
        )
        with (
            tc.tile_pool(name="consts", bufs=1) as consts,
            tc.tile_pool(name="stage", bufs=stage_bufs) as stage,
            tc.tile_pool(name="xt", bufs=16) as xt_pool,
            tc.tile_pool(name="wt", bufs=wt_bufs) as wt_pool,
            tc.tile_pool(name="outp", bufs=out_bufs) as outp,
            tc.tile_pool(name="psum", bufs=psum_bufs, space="PSUM") as psum,
            tc.tile_pool(name="psb", bufs=1, space="PSUM") as psb,
        ):
            ones = consts.tile([P, P], fp32, name="ones")
            nc.vector.memset(ones[:], 1.0)
            if mm_const_rhs:
                wt_const = consts.tile([P, 2, KT, P], bf16, name="wt_const")
                nc.vector.memset(wt_const[:], 1.0)
            racc = consts.tile([P, 4], fp32, name="racc")
            red1 = consts.tile([P, 1], fp32, name="red1")
            ccs = consts.tile([P, 1], fp32, name="ccs")
            scale_t = consts.tile([P, 1], fp32, name="scale_t")

            # ---- scale: |W| partial over this core's row slice, AllReduce ----
            if no_scale:
                nc.vector.memset(scale_t[:], 0.015)
            for rt in range(4 if not no_scale else 0):
                st = stage.tile([P, K], bf16, tag="stage", name="red_st")
                nc.gpsimd.dma_start(st[:], ws_d[rt * P : (rt + 1) * P, :])
                nc.vector.tensor_reduce(
                    racc[:, rt : rt + 1],
                    st[:],
                    axis=mybir.AxisListType.X,
                    op=mybir.AluOpType.add,
                    apply_absolute_value=True,
                )
            if not no_scale:
                nc.vector.tensor_reduce(
                    red1[:], racc[:], axis=mybir.AxisListType.X, op=mybir.AluOpType.add
                )
                nc.sync.dma_start(cc_in[:], red1[:])
                nc.gpsimd.collective_compute(
                    "AllReduce",
                    mybir.AluOpType.add,
                    replica_groups=[list(range(n_cores))],
                    ins=[cc_in[:]],
                    outs=[cc_out[:]],
                )
                nc.sync.dma_start(ccs[:], cc_out[:])
                ps1 = psb.tile([P, 1], fp32, name="ps1")
                nc.tensor.matmul(ps1[:], ones[:], ccs[:], start=True, stop=True)
                nc.scalar.mul(scale_t[:], ps1[:], 1.0 / (float(N) * float(K)))

            for _rep in range(reps):
                # ---- W chunks 0,1 first so chunk-0 prep overlaps x load ----
                wtcs = {}
                def prep_chunk(c):
                    # sign commutes with transpose: apply it on the 2D stage
                    # tile so the transposed wt chunk is written only by the
                    # xbar DMA (ACT in-place on the transposed tile measured
                    # pathologically slow on HW).
                    wtc = wt_pool.tile([P, 2, KT, P], bf16, tag="wt", name="wtc")
                    if not no_wstream:
                        for sub in range(2):
                            rt = 2 * c + sub
                            st = stage.tile([P, K], bf16, tag="stage", name="w_st")
                            nc.gpsimd.dma_start(st[:], w_d[rt * P : (rt + 1) * P, :])
                            if not no_sign:
                                if sign_as_copy:
                                    nc.scalar.copy(st[:], st[:])
                                elif sign_on_dve:
                                    nc.vector.tensor_scalar(
                                        out=st[:], in0=st[:],
                                        scalar1=0.0, scalar2=None,
                                        op0=mybir.AluOpType.is_ge,
                                    )
                                    nc.vector.tensor_scalar(
                                        out=st[:], in0=st[:],
                                        scalar1=2.0, scalar2=-1.0,
                                        op0=mybir.AluOpType.mult,
                                        op1=mybir.AluOpType.add,
                                    )
                                else:
                                    nc.scalar.sign(st[:], st[:])
                            nc.sync.dma_start(wtc[:, sub], st[:], transpose=True)
                    else:
                        nc.vector.memset(wtc[:], 1.0)
                    return wtc
                for c in range(2):
                    wtcs[c] = prep_chunk(c)

                # ---- x: cast-DMA to bf16, xbar-transpose into XT tiles ----
                xts = []
                for mt in range(MT):
                    xt = xt_pool.tile([P, KT, P], bf16, tag="xt", name="xt")
                    if not no_xprep:
                        st = stage.tile([P, K], bf16, tag="stage", name="x_st")
                        nc.gpsimd.dma_start(st[:], x_d[mt * P : (mt + 1) * P, :])
                        nc.sync.dma_start(xt[:], st[:], transpose=True)
                    else:
                        if mt == 0:
                            nc.vector.memset(xt[:], 0.5)
                    xts.append(xt)

                # ---- main chunk loop ----
                for c in range(NCH):
                    if c in wtcs:
                        wtc = wtcs[c]
                    else:
                        wtc = prep_chunk(c)
                    for mt in range(MT):
                        ps = psum.tile([P, CH], fp32, tag="ps", name="ps")
                        rhs = wt_const[:, :, :, :] if mm_const_rhs else wtc[:, :, :, :]
                        if not no_matmul:
                            for k in range(KT):
                                nc.tensor.matmul(
                                    ps[:],
                                    xts[mt][:, k, :],
                                    rhs[:, :, k, :],
                                    start=(k == 0),
                                    stop=(k == KT - 1),
                                )
                        else:
                            nc.vector.memset(ps[:], 0.0)
                        ob = outp.tile([P, CH], fp32, tag="ob", name="ob")
                        nc.scalar.activation(
                            ob[:],
                            ps[:],
                            mybir.ActivationFunctionType.Copy,
                            scale=scale_t[:],
                        )
                        if not no_store:
                            nc.sync.dma_start(
                                o_d[mt * P : (mt + 1) * P, c * CH : (c + 1) * CH], ob[:]
                            )

        cc_in_free()
        cc_out_free()

    nc.compile()
    return nc


def _get_runner(n_cores=8, reps=1):
    key = (n_cores, reps)
    if key not in _CACHE:
        nc = _build_program(n_cores, reps=reps)
        _CACHE[key] = _Runner(nc, n_cores)
    return _CACHE[key]


class _Runner:
    """Holds the compiled program and the jitted PJRT callable so repeat
    invocations skip retracing/recompiling."""

    def __init__(self, nc, n_cores):
        import jax
        import concourse.mybir as mybir
        import concourse.bass2jax as b2j

        self.n_cores = n_cores
        self.nc = nc
        captured = {}
        orig_jit = jax.jit

        def spy_jit(fn, **kw):
            jitted = orig_jit(fn, **kw)
            captured["fn"] = jitted
            return jitted

        self.in_names = []
        self.out_names = []
        self.out_shapes = {}
        in_specs = {}
        partition_name = nc.partition_id_tensor.name if nc.partition_id_tensor else None
        for alloc in nc.m.functions[0].allocations:
            if not isinstance(alloc, mybir.MemoryLocationSet):
                continue
            name = alloc.memorylocations[0].name
            if alloc.kind == "ExternalInput" and name != partition_name:
                self.in_names.append(name)
                in_specs[name] = (tuple(alloc.tensor_shape), mybir.dt.np(alloc.dtype))
            elif alloc.kind == "ExternalOutput":
                self.out_names.append(name)
                self.out_shapes[name] = (
                    tuple(alloc.tensor_shape),
                    mybir.dt.np(alloc.dtype),
                )

        b2j.jax.jit = spy_jit
        try:
            dummy = [
                {n: np.zeros(s, d) for n, (s, d) in in_specs.items()}
                for _ in range(n_cores)
            ]
            b2j.run_bass_via_pjrt(nc, dummy, n_cores=n_cores)
        finally:
            b2j.jax.jit = orig_jit
        assert "fn" in captured
        self.fn = captured["fn"]

    def run(self, in_maps):
        import jax

        args = []
        for name in self.in_names:
            args.append(np.concatenate([np.asarray(m[name]) for m in in_maps], axis=0))
        for name in self.out_names:
            shape, d = self.out_shapes[name]
            args.append(np.zeros((self.n_cores * shape[0], *shape[1:]), d))
        out = self.fn(*args)
        jax.block_until_ready(out)
        res = []
        for c in range(self.n_cores):
            d = {}
            for i, name in enumerate(self.out_names):
                shape, _ = self.out_shapes[name]
                d[name] = np.asarray(out[i]).reshape(self.n_cores, *shape)[c]
            res.append(d)
        return res


def kernel(x: np.ndarray, weight: np.ndarray) -> np.ndarray:
    assert x.shape == (8, M, K) and weight.shape == (N, K)
    x = np.ascontiguousarray(x, dtype=np.float32)
    weight = np.ascontiguousarray(weight, dtype=np.float32)
    runner = _get_runner(8)
    in_maps = [
        {
            "x": x[b],
            "weight": weight,
            "wslice": weight[b * RSLICE : (b + 1) * RSLICE, :],
        }
        for b in range(8)
    ]
    res = runner.run(in_maps)
    return np.stack([res[b]["out"] for b in range(8)], axis=0)



# revision 3
# speedup vs baseline: 57.5255x; 57.5255x over previous
"""BinaryLinear forward for Trainium2, 8-core SPMD.

Reference computation:
    scale = mean(|W|)                    # scalar over full W
    out   = x @ (sign(W) * scale).T      # x [8, 2048, 4096], W [4096, 4096]

Sharding: data-parallel over the batch dim (8 batches -> 8 cores).  W is
sharded 8-ways on the wire (each core uploads one 512-row slice); sign(W)
is applied to the shard on-device and the full sign(W) is reassembled with
an on-chip AllGather, so no host-side replication.  scale comes from a
|W|-partial per shard + AllReduce.  Device algorithm per core:
  - x m-tiles are xbar-transposed into k-major XT tiles (lhsT layout);
    sign(W) row-tiles stream from the AllGather buffer and are
    xbar-transposed into k-major rhs chunks of 512 out-features.
  - PE does K=128-deep accumulating matmuls into a full PSUM bank
    (psum[128, 512] over 32 k-tiles); scale is folded into the psum
    eviction (ACT copy with per-partition scale).
  - m-tiles run in two halves of 8 so resident XT + double-buffered W
    chunks fit SBUF; W streams twice (64 MB), well under the PE-bound
    ~0.9 ms.  Measured device time ~1.08 ms/iteration.

The wire dtype is chosen at first call by a bandwidth probe (_probe_
wire_fp32): over a slow tunnel everything crosses in bf16 (host casts,
half the bytes); over local PCIe everything stays fp32 (zero host-side
passes, device casts via DVE).  Override with KERNEL_WIRE=fp32|bf16.
"""

import numpy as np

P = 128
M = 2048  # rows per core (one batch)
K = 4096  # in_features
N = 4096  # out_features
MT = M // P  # 16 m-tiles
KT = K // P  # 32 k-tiles
CH = 512  # out_features per chunk (one PSUM bank at fp32)
NCH = N // CH  # 8 chunks
NSUB = CH // P  # 4 row-tiles of W per chunk
RSLICE = N // 8  # 512 rows of W per core shard
MHALF = MT // 2  # 8 m-tiles per half

_CACHE = {}


def _build_program(
    n_cores,
    reps=1,
    stage_bufs=3,
    wt_bufs=2,
    psum_bufs=6,
    out_bufs=3,
    hw_loop=0,
    wire_fp32=False,
):
    """wire_fp32=False: x/wsh/out cross the wire in bf16 (host casts, fewest
    bytes -- best over a slow link).  wire_fp32=True: everything fp32 on the
    wire, the device does the bf16 casts (zero host work -- best over PCIe).
    """
    import concourse.bass as bass
    import concourse.mybir as mybir
    import concourse.tile as tile
    from concourse import bacc

    # re-derive tiling from module globals so tests can shrink the dims
    MT = M // P
    KT = K // P
    NCH = N // CH
    NSUB = CH // P
    RSLICE = N // n_cores
    MHALF = max(MT // 2, 1)
    NHALF = (MT + MHALF - 1) // MHALF  # number of m-halves (2, or 1 when tiny)
    RTS = RSLICE // P  # row-tiles of the shard for the scale reduce

    fp32 = mybir.dt.float32
    bf16 = mybir.dt.bfloat16
    wire = fp32 if wire_fp32 else bf16

    nc = bacc.Bacc("TRN2", target_bir_lowering=False, debug=False, num_devices=n_cores)

    x_d = nc.dram_tensor("x", [M, K], wire, kind="ExternalInput").ap()
    w_d = nc.dram_tensor("wsh", [RSLICE, K], wire, kind="ExternalInput").ap()
    o_d = nc.dram_tensor("out", [M, N], wire, kind="ExternalOutput").ap()

    with tile.TileContext(nc) as tc:
        xt_bufs = MHALF if wire_fp32 else MHALF + 2
        if wire_fp32:
            stage_bufs = min(stage_bufs, 2)
        with (
            tc.tile_pool(name="dramp", bufs=1, space="DRAM") as dramp,
            tc.tile_pool(name="consts", bufs=1) as consts,
            tc.tile_pool(name="stage", bufs=stage_bufs) as stage,
            tc.tile_pool(name="st32p", bufs=2) as st32p,
            tc.tile_pool(name="xt", bufs=xt_bufs) as xt_pool,
            tc.tile_pool(name="wt", bufs=wt_bufs) as wt_pool,
            tc.tile_pool(name="outp", bufs=out_bufs) as outp,
            tc.tile_pool(name="psum", bufs=psum_bufs, space="PSUM") as psum,
            tc.tile_pool(name="psb", bufs=1, space="PSUM") as psb,
        ):
            ones = consts.tile([P, P], fp32, name="ones")
            nc.vector.memset(ones[:], 1.0)
            racc = consts.tile([P, RTS], fp32, name="racc")
            red1 = consts.tile([P, 1], fp32, name="red1")
            ccs = consts.tile([P, 1], fp32, name="ccs")
            scale_t = consts.tile([P, 1], fp32, name="scale_t")

            for _rep in range(reps):
                # Shared DRAM tensors may have only one writer instruction,
                # so the collective buffers are per-rep tiles (the pool slot
                # is reused; the tensors are distinct).
                cc_in = dramp.tile([P, 1], fp32, tag="cc_in", name="cc_in")
                cc_out = dramp.tile(
                    [P, 1], fp32, addr_space="Shared", tag="cc_out", name="cc_out"
                )
                ag_in = dramp.tile([RSLICE, K], bf16, tag="ag_in", name="ag_in")
                ag_out = dramp.tile(
                    [N, K], bf16, addr_space="Shared", tag="ag_out", name="ag_out"
                )
                # ---- load shard once: |W| partial for scale, sign, ship off
                # for AllGather (so sign runs 4x instead of 32x) ----
                for rt in range(RTS):
                    if wire_fp32:
                        st32 = st32p.tile([P, K], fp32, tag="st32", name="sh32")
                        nc.sync.dma_start(st32[:], w_d[rt * P : (rt + 1) * P, :])
                        red_src = st32
                        st = stage.tile([P, K], bf16, tag="stage", name="sh_st")
                        nc.vector.tensor_copy(st[:], st32[:])
                    else:
                        st = stage.tile([P, K], bf16, tag="stage", name="sh_st")
                        nc.sync.dma_start(st[:], w_d[rt * P : (rt + 1) * P, :])
                        red_src = st
                    nc.vector.tensor_reduce(
                        racc[:, rt : rt + 1],
                        red_src[:],
                        axis=mybir.AxisListType.X,
                        op=mybir.AluOpType.add,
                        apply_absolute_value=True,
                    )
                    nc.scalar.sign(st[:], st[:])
                    nc.sync.dma_start(ag_in[rt * P : (rt + 1) * P, :], st[:])
                nc.gpsimd.collective_compute(
                    "AllGather",
                    mybir.AluOpType.bypass,
                    replica_groups=[list(range(n_cores))],
                    ins=[ag_in[:, :]],
                    outs=[ag_out[:, :]],
                )

                # ---- scale: AllReduce the |Wshard| partial over cores ----
                nc.vector.tensor_reduce(
                    red1[:], racc[:], axis=mybir.AxisListType.X, op=mybir.AluOpType.add
                )
                nc.sync.dma_start(cc_in[:], red1[:])
                nc.gpsimd.collective_compute(
                    "AllReduce",
                    mybir.AluOpType.add,
                    replica_groups=[list(range(n_cores))],
                    ins=[cc_in[:]],
                    outs=[cc_out[:]],
                )
                nc.sync.dma_start(ccs[:], cc_out[:])
                ps1 = psb.tile([P, 1], fp32, name="ps1")
                nc.tensor.matmul(ps1[:], ones[:], ccs[:], start=True, stop=True)
                nc.scalar.mul(scale_t[:], ps1[:], 1.0 / (float(N) * float(K)))

                def prep_chunk(c):
                    # ag_out already holds sign(W) in bf16 (+-1 exact):
                    # just load row-tiles and xbar-transpose into k-major.
                    wtc = wt_pool.tile([P, NSUB, KT, P], bf16, tag="wt", name="wtc")
                    for sub in range(NSUB):
                        rt = NSUB * c + sub
                        st = stage.tile([P, K], bf16, tag="stage", name="w_st")
                        nc.sync.dma_start(st[:], ag_out[rt * P : (rt + 1) * P, :])
                        nc.sync.dma_start(wtc[:, sub], st[:], transpose=True)
                    return wtc

                def body():
                    for half in range(NHALF):
                        # ---- x: (cast then) xbar-transpose into k-major ----
                        xts = []
                        for i in range(MHALF):
                            mt = half * MHALF + i
                            xt = xt_pool.tile([P, KT, P], bf16, tag="xt", name="xt")
                            if wire_fp32:
                                x32 = st32p.tile([P, K], fp32, tag="st32", name="x32")
                                nc.sync.dma_start(
                                    x32[:], x_d[mt * P : (mt + 1) * P, :]
                                )
                                xb = stage.tile([P, K], bf16, tag="stage", name="xb")
                                nc.vector.tensor_copy(xb[:], x32[:])
                                nc.sync.dma_start(xt[:], xb[:], transpose=True)
                            else:
                                nc.sync.dma_start(
                                    xt[:], x_d[mt * P : (mt + 1) * P, :], transpose=True
                                )
                            xts.append(xt)

                        wtcs = {0: prep_chunk(0)}
                        for c in range(NCH):
                            wtc = wtcs.pop(c) if c in wtcs else prep_chunk(c)
                            for i in range(MHALF):
                                mt = half * MHALF + i
                                ps = psum.tile([P, CH], fp32, tag="ps", name="ps")
                                for kt in range(KT):
                                    nc.tensor.matmul(
                                        ps[:],
                                        xts[i][:, kt, :],
                                        wtc[:, :, kt, :],
                                        start=(kt == 0),
                                        stop=(kt == KT - 1),
                                    )
                                ob = outp.tile([P, CH], wire, tag="ob", name="ob")
                                nc.scalar.activation(
                                    ob[:],
                                    ps[:],
                                    mybir.ActivationFunctionType.Copy,
                                    scale=scale_t[:],
                                )
                                nc.sync.dma_start(
                                    o_d[mt * P : (mt + 1) * P, c * CH : (c + 1) * CH],
                                    ob[:],
                                )

                if hw_loop:
                    with tc.For_i(0, hw_loop):
                        body()
                else:
                    body()

    nc.compile()
    return nc


def _get_runner(n_cores=8, reps=1, wire_fp32=False):
    key = (n_cores, reps, wire_fp32)
    if key not in _CACHE:
        nc = _build_program(n_cores, reps=reps, wire_fp32=wire_fp32)
        _CACHE[key] = _Runner(nc, n_cores)
    return _CACHE[key]


def _probe_wire_fp32():
    """Decide the wire dtype once per process: fp32 when the link to the
    devices is fast (PCIe-local -- host casts would dominate), bf16 when it
    is slow (remote tunnel -- wire bytes dominate).  Cached in _CACHE."""
    if "wire_fp32" not in _CACHE:
        import os
        import time
        import jax

        env = os.environ.get("KERNEL_WIRE", "")
        if env in ("fp32", "bf16"):
            _CACHE["wire_fp32"] = env == "fp32"
            return _CACHE["wire_fp32"]
        try:
            dev = jax.devices()[0]
            warm = jax.device_put(np.zeros((256, 1024), np.uint8), dev)
            warm.block_until_ready()
            buf = np.empty((32, 1024, 1024), np.uint8)
            t0 = time.perf_counter()
            jax.device_put(buf, dev).block_until_ready()
            bw = buf.nbytes / (time.perf_counter() - t0)  # B/s
            _CACHE["wire_fp32"] = bw > 1.5e9
        except Exception:
            _CACHE["wire_fp32"] = False
    return _CACHE["wire_fp32"]


class _Runner:
    """Holds the compiled program and a NON-donating jitted PJRT callable.

    The upstream run_bass_via_pjrt jit donates the zero output buffers, so
    each call would have to re-upload them.  We re-jit the same shard_map
    wrapper without donation and keep the zero buffers resident on device
    (created with jnp.zeros under out_shardings -- no wire traffic), so a
    steady-state call uploads only the real inputs.
    """

    def __init__(self, nc, n_cores):
        import jax
        import jax.numpy as jnp
        from jax.sharding import Mesh, NamedSharding, PartitionSpec
        import concourse.mybir as mybir
        import concourse.bass2jax as b2j

        self.n_cores = n_cores
        self.nc = nc
        captured = {}
        orig_jit = jax.jit

        def spy_jit(fn, **kw):
            captured["wrapper"] = fn
            jitted = orig_jit(fn, **kw)
            captured["fn"] = jitted
            return jitted

        self.in_names = []
        self.out_names = []
        self.out_shapes = {}
        in_specs = {}
        partition_name = nc.partition_id_tensor.name if nc.partition_id_tensor else None
        for alloc in nc.m.functions[0].allocations:
            if not isinstance(alloc, mybir.MemoryLocationSet):
                continue
            name = alloc.memorylocations[0].name
            if alloc.kind == "ExternalInput" and name != partition_name:
                self.in_names.append(name)
                in_specs[name] = (tuple(alloc.tensor_shape), mybir.dt.np(alloc.dtype))
            elif alloc.kind == "ExternalOutput":
                self.out_names.append(name)
                self.out_shapes[name] = (
                    tuple(alloc.tensor_shape),
                    mybir.dt.np(alloc.dtype),
                )

        b2j.jax.jit = spy_jit
        try:
            dummy = [
                {n: np.zeros(s, d) for n, (s, d) in in_specs.items()}
                for _ in range(n_cores)
            ]
            b2j.run_bass_via_pjrt(nc, dummy, n_cores=n_cores)
        finally:
            b2j.jax.jit = orig_jit
        assert "wrapper" in captured
        self.fn = jax.jit(captured["wrapper"], keep_unused=True)

        devices = jax.devices()[:n_cores]
        mesh = Mesh(np.asarray(devices), ("core",))
        self.zeros_dev = []
        for name in self.out_names:
            shape, d = self.out_shapes[name]
            gshape = (n_cores * shape[0], *shape[1:])
            spec = PartitionSpec("core", *([None] * (len(gshape) - 1)))
            sharding = NamedSharding(mesh, spec)
            z = jax.jit(
                lambda gshape=gshape, d=d: jnp.zeros(gshape, d),
                out_shardings=sharding,
            )()
            z.block_until_ready()
            self.zeros_dev.append(z)

    def run(self, full_maps):
        """full_maps: dict name -> already-concatenated global array."""
        import jax

        args = [full_maps[name] for name in self.in_names]
        out = self.fn(*args, *self.zeros_dev)
        jax.block_until_ready(out)
        return {name: np.asarray(out[i]) for i, name in enumerate(self.out_names)}


def kernel(x: np.ndarray, weight: np.ndarray) -> np.ndarray:
    assert x.shape == (8, M, K) and weight.shape == (N, K)
    wire_fp32 = _probe_wire_fp32()
    runner = _get_runner(8, wire_fp32=wire_fp32)
    if wire_fp32:
        # zero host-side passes: views in, fp32 straight back out
        full = {
            "x": np.asarray(x, dtype=np.float32).reshape(8 * M, K),
            "wsh": np.asarray(weight, dtype=np.float32),
        }
        res = runner.run(full)
        return res["out"].reshape(8, M, N)
    import ml_dtypes

    xb = np.asarray(x, dtype=np.float32).astype(ml_dtypes.bfloat16)
    wb = np.asarray(weight, dtype=np.float32).astype(ml_dtypes.bfloat16)
    full = {
        "x": xb.reshape(8 * M, K),
        "wsh": wb,  # [4096, 4096] == 8 shards of [512, 4096] stacked
    }
    res = runner.run(full)
    return res["out"].astype(np.float32).reshape(8, M, N)
